# revision 14
# baseline (speedup 1.0000x reference)
"""Trainium2 Bass kernel: batched attention-distribution forward.

Computes, for x:[B,S,F], Wq/Wk:[F,D], bq/bk:[D]:
    q = x@Wq + bq ; k = x@Wk + bk
    qkt = q @ k^T                    # [B,S,S]
    dist = softmax(qkt / rowmax(qkt))

Sharding: 8 NeuronCores, core c -> batch c//2, query-row half c%2.
Each core emits a [2048, 4096] f32 slab (33.5 MB) -> memory-bound on the
HBM write (~358 GB/s/core).

Per-core pipeline, per 128-row tile (PSUM = 4 chunks of [128,1024], so
each chunk is released right after its exp and the next tile's matmuls
stream in behind):
  PE    : qkt chunk into PSUM (2x N=512 matmuls)
  DVE   : reduce_max per chunk -> mvec column; combine + reciprocal
  ACT   : per-chunk Exp(scale=1/M, bias=-1) PSUM->SBUF with
          accum_out -> svec column (softmax is shift invariant and
          rowmax > 0 for this regime, so exp(z-1)/sum == reference)
  DVE/ACT: normalize (x *= 1/sum), column-split to balance engines
  DMA   : 2 MB tile store

Host-side prep is layout only (transpose x, append ones-row so the bias
rides inside the matmul contraction); every FLOP runs on device.
"""

from contextlib import ExitStack

import numpy as np

import concourse.bacc as bacc
import concourse.bass as bass
import concourse.mybir as mybir
import concourse.tile as tile
from concourse.bass_utils import run_bass_kernel_spmd

B, S, F, D = 4, 4096, 33, 64
NCORES = 8
HALF = S // 2        # query rows per core
PT = 128             # rows per tile
NT = HALF // PT      # 16 tiles
FA = F + 1           # features + ones-row (bias folded into matmul)
CW = 1024            # PSUM chunk width (2 banks); 4 chunks per row-tile
NCH = S // CW        # 4
DVE_COLS = 1664      # normalize: columns on DVE; rest on ACT (engine balance)

F32 = mybir.dt.float32


def build_bass() -> bass.Bass:
    nc = bacc.Bacc(trn_type="TRN2")
    # xaw = [x[b]^T aug | Wk aug] ; xqw = [x[b]^T aug (this half) | Wq aug]
    # packed so each core input is ONE dma (single DMA-sem lane -> the
    # matmuls that read them need only one sync wait; LDWEIGHTS has a
    # 1-wait budget).
    xaw = nc.declare_dram_parameter("xaw", [FA, S + D], F32, isOutput=False)
    xqw = nc.declare_dram_parameter("xqw", [FA, HALF + D], F32, isOutput=False)
    out = nc.declare_dram_parameter("out", [HALF, S], F32, isOutput=True)

    add = mybir.AluOpType.add
    amax = mybir.AluOpType.max
    Exp = mybir.ActivationFunctionType.Exp

    with tile.TileContext(nc) as tc, ExitStack() as ctx:
        singles = ctx.enter_context(tc.tile_pool(name="singles", bufs=1))
        psum = ctx.enter_context(tc.tile_pool(name="psum", bufs=4, space="PSUM"))
        e_pool = ctx.enter_context(tc.tile_pool(name="e", bufs=3))
        stats = ctx.enter_context(tc.tile_pool(name="stats", bufs=6))

        # ---- load inputs (one DMA per packed tensor) ----
        xaw_sb = singles.tile([FA, S + D], F32)
        nc.sync.dma_start(out=xaw_sb[:, :], in_=xaw[:, :])
        xqw_sb = singles.tile([FA, HALF + D], F32)
        nc.sync.dma_start(out=xqw_sb[:, :], in_=xqw[:, :])
        neg1 = singles.tile([PT, 1], F32)
        nc.vector.memset(neg1[:, :], -1.0)

        # ---- projections: qT = (xq^T @ Wq)^T, kT likewise ----
        qT = singles.tile([D, HALF], F32)
        kT = singles.tile([D, S], F32)

        pq = psum.tile([PT, CW], F32, tag="ps")
        for j in range(2):
            nc.tensor.matmul(
                pq[0:D, j * 512:(j + 1) * 512],
                lhsT=xqw_sb[:, HALF:HALF + D],
                rhs=xqw_sb[:, j * 512:(j + 1) * 512],
                start=True, stop=True,
            )
        pq2 = psum.tile([PT, CW], F32, tag="ps")
        for j in range(2):
            nc.tensor.matmul(
                pq2[0:D, j * 512:(j + 1) * 512],
                lhsT=xqw_sb[:, HALF:HALF + D],
                rhs=xqw_sb[:, CW + j * 512:CW + (j + 1) * 512],
                start=True, stop=True,
            )
        nc.vector.tensor_copy(qT[:, 0:CW], pq[0:D, :])
        nc.vector.tensor_copy(qT[:, CW:HALF], pq2[0:D, :])

        # kT copies also on DVE: every SBUF tensor feeding PE is written by
        # DVE, so main-loop matmuls carry few sync waits.
        for h in range(NCH):
            pk = psum.tile([PT, CW], F32, tag="ps")
            for j in range(2):
                c0 = h * CW + j * 512
                nc.tensor.matmul(
                    pk[0:D, j * 512:(j + 1) * 512],
                    lhsT=xaw_sb[:, S:S + D],
                    rhs=xaw_sb[:, c0:c0 + 512],
                    start=True, stop=True,
                )
            nc.vector.tensor_copy(kT[:, h * CW:(h + 1) * CW], pk[0:D, :])

        # ---- main loop: one 128-query-row tile at a time ----
        for t in range(NT):
            lhsT = qT[:, t * PT:(t + 1) * PT]
            chunks = []
            mvec = stats.tile([PT, NCH], F32, tag="mvec")
            for c in range(NCH):
                ps = psum.tile([PT, CW], F32, tag="ps")
                for j in range(2):
                    c0 = c * CW + j * 512
                    nc.tensor.matmul(
                        ps[:, j * 512:(j + 1) * 512],
                        lhsT=lhsT,
                        rhs=kT[:, c0:c0 + 512],
                        start=True, stop=True,
                    )
                nc.vector.reduce_max(
                    mvec[:, c:c + 1], ps[:, :], axis=mybir.AxisListType.X
                )
                chunks.append(ps)

            m = stats.tile([PT, 1], F32, tag="m")
            nc.vector.reduce_max(m[:, 0:1], mvec[:, :], axis=mybir.AxisListType.X)
            rM = stats.tile([PT, 1], F32, tag="rM")
            nc.vector.reciprocal(rM[:, 0:1], m[:, 0:1])

            e = e_pool.tile([PT, S], F32)
            svec = stats.tile([PT, NCH], F32, tag="svec")
            for c in range(NCH):
                nc.scalar.activation(
                    out=e[:, c * CW:(c + 1) * CW],
                    in_=chunks[c][:, :],
                    func=Exp,
                    bias=neg1[:, 0:1],
                    scale=rM[:, 0:1],
                    accum_out=svec[:, c:c + 1],
                )
            ssum = stats.tile([PT, 1], F32, tag="ssum")
            nc.vector.reduce_sum(
                ssum[:, 0:1], svec[:, :], axis=mybir.AxisListType.X
            )
            rs = stats.tile([PT, 1], F32, tag="rs")
            nc.vector.reciprocal(rs[:, 0:1], ssum[:, 0:1])

            nc.vector.tensor_scalar_mul(
                e[:, 0:DVE_COLS], e[:, 0:DVE_COLS], rs[:, 0:1]
            )
            nc.scalar.mul(e[:, DVE_COLS:S], e[:, DVE_COLS:S], rs[:, 0:1])

            nc.sync.dma_start(out=out[t * PT:(t + 1) * PT, :], in_=e[:, :])

    nc.compile()
    return nc


_NC = None


def _get_nc() -> bass.Bass:
    global _NC
    if _NC is None:
        _NC = build_bass()
    return _NC


def prepare_in_maps(inputs: dict) -> list[dict]:
    x = np.ascontiguousarray(np.asarray(inputs["x"], dtype=np.float32))
    Wq = np.asarray(inputs["Wq"], dtype=np.float32)
    bq = np.asarray(inputs["bq"], dtype=np.float32)
    Wk = np.asarray(inputs["Wk"], dtype=np.float32)
    bk = np.asarray(inputs["bk"], dtype=np.float32)

    wq_aug = np.concatenate([Wq, bq[None, :]], axis=0)
    wk_aug = np.concatenate([Wk, bk[None, :]], axis=0)

    in_maps = []
    xaw_cache = {}
    for c in range(NCORES):
        b, h = c // 2, c % 2
        if b not in xaw_cache:
            xaw = np.empty((FA, S + D), np.float32)
            xaw[:F, :S] = x[b].T
            xaw[F, :S] = 1.0
            xaw[:, S:] = wk_aug
            xaw_cache[b] = xaw
        xaw = xaw_cache[b]
        xqw = np.empty((FA, HALF + D), np.float32)
        xqw[:, :HALF] = xaw[:, h * HALF:(h + 1) * HALF]
        xqw[:, HALF:] = wq_aug
        in_maps.append({"xaw": xaw, "xqw": xqw})
    return in_maps


def run(in_maps: list[dict], **kwargs):
    return run_bass_kernel_spmd(
        _get_nc(), in_maps, core_ids=list(range(NCORES)), **kwargs
    )


def assemble(results: list[dict]) -> np.ndarray:
    out = np.empty((B, S, S), np.float32)
    for c in range(NCORES):
        b, h = c // 2, c % 2
        out[b, h * HALF:(h + 1) * HALF, :] = results[c]["out"]
    return out


def kernel(**inputs) -> np.ndarray:
    res = run(prepare_in_maps(inputs))
    return assemble(res.results)
